# revision 18
# baseline (speedup 1.0000x reference)
"""Trainium2 Bass kernel for nn_MultiHeadAttention_73504070303932.

Multi-head causal attention with a learned per-head key scale on a shared
K=V projection:

    q  = (x @ w_q) / sqrt(d)          reshaped to (b, h, L, v)
    kv = x @ w_kv                     reshaped to (b, h, L, v)
    k  = kv * (1 + k_weights)
    y  = softmax(causal(q k^T)) @ kv
    out = y @ w_o

Shapes: x [4, 1024, 1024], w_q/w_kv/w_o [1024, 1024],
k_weights [1, 16, 1, 64]; h=16 heads of v=64.

Sharding (8 NeuronCores): data-parallel over batch (4) x tensor-parallel
over head halves (2), with NO device collective: core c handles batch
c//2 and heads (c%2)*8..+8, computes a full-shape PARTIAL output
projection out_c = y_half @ w_o[half rows, :], and the host sums the two
partials per batch while unsharding.  (A previous version exchanged y
halves with an in-kernel AllGather; collectives wedge the device when
re-executed inside a hardware loop, and the host-side add of two
[1024, 1024] partials is negligible.)

Performance model for this target (measured via microbenchmarks): each
engine executes its instruction queue serially; a *freshly streamed*
instruction costs ~50-80 us regardless of size, but re-executing the
same instruction inside a hardware For_i loop costs only ~1-10 us
(f32r matmul N=512 ~4 us, act exp [128,1024] ~6 us, DVE ~1 us; plain
f32 matmuls are ~13x slower than f32r, so everything stays f32r).
Engines run concurrently, so wall time ~= tensor-engine queue time once
streaming is amortized.  The kernel body is therefore built to be
wrapped in a single For_i for repeated execution (timing runs), and the
body itself minimizes tensor-engine instructions (384 matmuls: 128
projection + 96 QK + 96 AV + 64 output projection) and keeps every
other engine off the critical path:

  - projections compute q'^T / kv^T [hv, seq] with weight tiles
    stationary; (1 + k_weights)/sqrt(d) is folded into wq host-side;
    double-buffered 4-bank psum tiles so copy-out overlaps matmuls.
  - kv natural [seq, hv] (the AV stationary operand, with a 65th ones
    column so the softmax denominator accumulates for free in psum row
    64) is produced by a DRAM round trip: one write, one strided
    transpose-gather DMA into the persistent 65-column layout whose
    ones columns are initialized once outside the loop.
  - causal attention per head over 2-j-tile logit groups ([128, 1024]
    psum, double-buffered so the next group's QK matmuls overlap the
    current group's exp on the scalar engine); no max subtraction
    (logits are O(1) by construction); diagonal-straddling groups are
    masked post-exp with a 0/1 mask on the vector engine.
  - softmax normalization per head: reciprocal of psum row 64 (DVE),
    partition-broadcast of the reciprocal row (GpSimd engine, otherwise
    idle), one DVE multiply that also drains the AV psum into y^T.
  - output projection: y^T tiles stationary, w_o moving, 8 psum tiles
    of [128, 1024] rotating through 4 buffers; host adds the two
    partial outputs per batch.

Measured: relative error ~3e-4; per-iteration time (For_i-amortized)
~1.5-2.5 ms/iteration vs ~27 ms for the unrolled baseline.
"""

import math

import numpy as np

import concourse.bass as bass
import concourse.mybir as mybir
import concourse.tile as tile
from concourse import bacc
from concourse.bass_utils import run_bass_kernel_spmd

F32 = mybir.dt.float32
F32R = mybir.dt.float32r

N_CORES = 8
B, SEQ, D = 4, 1024, 1024
H, V = 16, 64
HL = 8          # heads per core
HV = HL * V     # 512 local feature dim
P = 128         # partitions
IB = 512        # i-block (query) width in the attention loop
NI = SEQ // IB  # 2 i-blocks
ND = D // P     # 8 d-tiles
NS = SEQ // P   # 8 seq j-tiles
NHV = HV // P   # 4 local hv tiles
KVC = V + 1     # kv columns per head incl the ones column
GJ = 2          # j-tiles per exp group ([128, GJ*IB] = 2 psum banks)

# set False to replace the GpSimd partition_broadcast in the softmax
# normalization with PE broadcast matmuls (fallback if unsupported)
USE_PBCAST = True


def build_program(n_iters: int = 1, use_loop: bool = True):
    """Build the SPMD program (same for all 8 cores). Returns compiled nc.

    n_iters > 1 repeats the body for on-device timing runs; with
    use_loop=True the repeat is a hardware For_i loop (instructions are
    streamed once and re-executed, ~10x cheaper per iteration than
    streaming fresh instructions).
    """
    nc = bacc.Bacc(trn_type="TRN2", target_bir_lowering=False, debug=False,
                   num_devices=N_CORES)

    xT = nc.dram_tensor("xT", [D, SEQ], F32R, kind="ExternalInput").ap()
    wqkv = nc.dram_tensor("wqkv", [D, 2 * HV], F32R, kind="ExternalInput").ap()
    woh = nc.dram_tensor("woh", [HV, D], F32R, kind="ExternalInput").ap()
    # 0/1 causal masks for the 4 j-tile offsets within an i-block
    maskq = nc.dram_tensor("maskq", [P, 4 * IB], F32, kind="ExternalInput").ap()
    out = nc.dram_tensor("out", [SEQ, D], F32, kind="ExternalOutput").ap()

    Exp = mybir.ActivationFunctionType.Exp

    with tile.TileContext(nc) as tc:
        with (
            tc.tile_pool(name="consts", bufs=1) as consts,
            tc.tile_pool(name="dram", bufs=1, space="DRAM") as dram,
        ):
            maskq_sb = consts.tile([P, 4 * IB], F32)
            ones8_f = consts.tile([P, NS * HL], F32)
            ones64_f = consts.tile([1, V], F32)
            ones64_r = consts.tile([1, V], F32R)
            # kv natural incl. ones columns: persistent so the ones are
            # written once; the per-iteration gather only refreshes the
            # 64 data columns of each (j-tile, head) slot
            kvn = consts.tile([P, NS * HL * KVC], F32R)
            nc.sync.dma_start(maskq_sb[:], maskq[:])
            nc.vector.memset(ones8_f[:], 1.0)
            nc.vector.memset(ones64_f[:], 1.0)
            nc.vector.tensor_copy(ones64_r[:], ones64_f[:])
            nc.vector.tensor_copy(
                kvn[:].rearrange("p (t c) -> p t c", c=KVC)[:, :, V:V + 1],
                ones8_f[:].rearrange("p (t o) -> p t o", o=1))

            kv_dram = dram.tile([HV, SEQ], F32R)

            if use_loop:
                # always a For_i, even for n_iters=1, so 1-iter and N-iter
                # programs have identical static structure and the timing
                # difference is purely N-1 body re-executions
                with tc.For_i(0, n_iters):
                    _one_iter(nc, tc, 0, xT, wqkv, woh, out,
                              maskq_sb, ones64_r, kvn, kv_dram, Exp)
            else:
                for it in range(n_iters):
                    _one_iter(nc, tc, it, xT, wqkv, woh, out,
                              maskq_sb, ones64_r, kvn, kv_dram, Exp)

    nc.compile()
    return nc


def _one_iter(nc, tc, it, xT, wqkv, woh, out,
              maskq_sb, ones64_r, kvn, kv_dram, Exp):
    with (
        tc.tile_pool(name=f"qkv{it}", bufs=1) as qkv,
        tc.tile_pool(name=f"ytp{it}", bufs=1) as ytp,
    ):
        # persistent on-core tensors for this iteration
        qkT = qkv.tile([P, 2 * NHV * SEQ], F32R, tag="qkT", name="qkT")
        qT = qkT[:, 0:NHV * SEQ]
        kvT = qkT[:, NHV * SEQ:2 * NHV * SEQ]
        wo_sb = qkv.tile([P, NHV * SEQ], F32R, tag="wos", name="wos")
        yT = ytp.tile([P, NHV * SEQ], F32R, tag="yTt", name="yTt")

        # ---- load x^T + weights, project q'^T and kv^T ----
        with (
            tc.tile_pool(name=f"xw{it}", bufs=1) as xw,
            tc.tile_pool(name=f"mmps{it}", bufs=2, space="PSUM") as mmps,
        ):
            xT_sb = xw.tile([P, ND * SEQ], F32R, tag="xTs", name="xTs")
            wqkv_sb = xw.tile([P, ND * 2 * HV], F32R, tag="wqs", name="wqs")
            # per-k-tile loads: the m=0 projection chain's k-th matmul only
            # depends on the k-th slices, so PE starts ~one-tile after the
            # first slices land instead of waiting for the full 8 MB
            xT3 = xT.rearrange("(k p) s -> p k s", p=P)
            wq3 = wqkv.rearrange("(k p) n -> p k n", p=P)
            for k in range(ND):
                nc.scalar.dma_start(
                    wqkv_sb[:, k * 2 * HV:(k + 1) * 2 * HV], wq3[:, k, :])
                nc.sync.dma_start(
                    xT_sb[:, k * SEQ:(k + 1) * SEQ], xT3[:, k, :])

            # q^T / kv^T: [hv-tile m, seq] = sum_k w[:, m]^T @ x^T;
            # (1+k_weights)/sqrt(d) is pre-folded into wq's columns
            for m in range(NHV):
                ps_qk = mmps.tile([P, 2048], F32, tag="mm", name="ps_qk")
                for n in range(SEQ // 512):
                    x_k0 = None
                    for k in range(ND):
                        x_k = xT_sb[:, k * SEQ + n * 512:k * SEQ + (n + 1) * 512]
                        wq_k = wqkv_sb[:, k * 2 * HV + m * P:
                                       k * 2 * HV + (m + 1) * P]
                        wkv_k = wqkv_sb[:, k * 2 * HV + HV + m * P:
                                        k * 2 * HV + HV + (m + 1) * P]
                        nc.tensor.matmul(ps_qk[:, n * 512:(n + 1) * 512],
                                         wq_k, x_k,
                                         start=(k == 0), stop=(k == ND - 1))
                        nc.tensor.matmul(
                            ps_qk[:, 1024 + n * 512:1024 + (n + 1) * 512],
                            wkv_k, x_k,
                            start=(k == 0), stop=(k == ND - 1))
                # one copy lands this m-tile's q and kv chunks
                nc.vector.tensor_copy(
                    qkT[:].rearrange("p (sel m s) -> p sel m s",
                                     sel=2, s=SEQ)[:, :, m, :],
                    ps_qk[:].rearrange("p (sel s) -> p sel s", s=SEQ))

        # wo is only needed by the output projection: load it on the ACT
        # engine's DMA queue after the wqkv tiles, overlapping attention
        nc.scalar.dma_start(wo_sb[:].rearrange("p (g n) -> p g n", n=D),
                            woh.rearrange("(g p) n -> p g n", p=P))

        # kv natural via DRAM round trip: write kv^T once, one strided
        # transpose-gather refreshing kvn's 64 data columns per slot
        nc.sync.dma_start(kv_dram.rearrange("(m p) s -> p m s", p=P),
                          kvT[:].rearrange("p (m s) -> p m s", s=SEQ))
        with tc.tile_pool(name=f"kvs{it}", bufs=1) as kvs:
            kvst = kvs.tile([P, NS * HV], F32R, tag="kvst", name="kvst")
            for t in range(NS):
                nc.sync.dma_start(
                    kvst[:, t * HV:(t + 1) * HV],
                    bass.AP(kv_dram.tensor, kv_dram.offset + t * P,
                            [[1, P], [SEQ, HV]]))
            nc.vector.tensor_copy(
                kvn[:].rearrange("p (t h c) -> p t h c",
                                 h=HL, c=KVC)[:, :, :, 0:V],
                kvst[:].rearrange("p (t h c) -> p t h c", h=HL, c=V))

        # ---- causal attention, head by head ----
        with (
            tc.tile_pool(name=f"st{it}", bufs=5) as stp,
            tc.tile_pool(name=f"sd{it}", bufs=2) as sdp,
            tc.tile_pool(name=f"rs{it}", bufs=4) as rsp,
            tc.tile_pool(name=f"ltps{it}", bufs=2, space="PSUM") as ltps,
            tc.tile_pool(name=f"yps{it}", bufs=2, space="PSUM") as yps,
        ):
            for u in range(HL // 2):         # head pair (2u, 2u+1)
                for hh in range(2):
                    h, r0 = 2 * u + hh, hh * V
                    ps_y = yps.tile([V + 1, NI * IB], F32, tag="y",
                                    name="ps_y")
                    sts = {}
                    for i in range(NI):
                        nj = (i + 1) * IB // P   # causal j-tiles
                        for g in range(nj // GJ):
                            ps_l = ltps.tile([P, GJ * IB], F32, tag="lt",
                                             name="ps_l")
                            for jo in range(GJ):
                                j = GJ * g + jo
                                nc.tensor.matmul(
                                    ps_l[:, jo * IB:(jo + 1) * IB],
                                    kvT[r0:r0 + V,
                                        u * SEQ + j * P:u * SEQ + (j + 1) * P],
                                    qT[r0:r0 + V,
                                       u * SEQ + i * IB:u * SEQ + (i + 1) * IB],
                                    start=True, stop=True)
                            st = stp.tile([P, GJ * IB], F32R, tag="st",
                                          name="st")
                            if g >= 2 * i:   # diagonal-straddling group
                                v = g - 2 * i
                                sd = sdp.tile([P, GJ * IB], F32, tag="sd",
                                              name="sd")
                                nc.scalar.activation(sd[:], ps_l[:], Exp)
                                nc.vector.tensor_tensor(
                                    st[:], sd[:],
                                    maskq_sb[:, v * GJ * IB:(v + 1) * GJ * IB],
                                    mybir.AluOpType.mult)
                            else:
                                nc.scalar.activation(st[:], ps_l[:], Exp)
                            sts[(i, g)] = st
                        for j in range(nj):
                            nc.tensor.matmul(
                                ps_y[0:V + 1, i * IB:(i + 1) * IB],
                                kvn[:, (j * HL + h) * KVC:
                                    (j * HL + h + 1) * KVC],
                                sts[(i, j // GJ)][:, (j % GJ) * IB:
                                                  (j % GJ + 1) * IB],
                                start=(j == 0), stop=(j == nj - 1))
                    # normalize: recip of the denominator row, broadcast
                    # across the 64 v-partitions on GpSimd, one multiply
                    # that also drains the AV psum into y^T
                    rr = rsp.tile([1, NI * IB], F32, tag="rr", name="rr")
                    with nc.allow_low_precision(reason="denom to f32"):
                        nc.vector.reciprocal(rr[:], ps_y[V:V + 1, :])
                    rB = rsp.tile([V, NI * IB], F32, tag="rB", name="rB")
                    if USE_PBCAST:
                        nc.gpsimd.partition_broadcast(rB[:], rr[:])
                    else:
                        with tc.tile_pool(name=f"b{it}_{h}", bufs=1,
                                          space="PSUM") as bps:
                            ps_b = bps.tile([V, NI * IB], F32, tag="b",
                                            name="ps_b")
                            for i in range(NI):
                                nc.tensor.matmul(ps_b[:, i * IB:(i + 1) * IB],
                                                 ones64_r[:],
                                                 rr[:, i * IB:(i + 1) * IB],
                                                 start=True, stop=True)
                            nc.vector.tensor_copy(rB[:], ps_b[:])
                    nc.vector.tensor_tensor(
                        yT[r0:r0 + V, u * SEQ:(u + 1) * SEQ],
                        ps_y[0:V, :], rB[:], mybir.AluOpType.mult)

        # ---- partial output projection: out = y_half^T.T @ wo_half ----
        with (
            tc.tile_pool(name=f"os{it}", bufs=1) as osp,
            tc.tile_pool(name=f"ops{it}", bufs=4, space="PSUM") as ops,
        ):
            o_sb = osp.tile([P, NS * D], F32, tag="osb", name="osb")
            for mt in range(NS):
                ps_o = ops.tile([P, D], F32, tag="om", name="ps_o")
                for n in range(D // 512):
                    for g in range(NHV):
                        nc.tensor.matmul(
                            ps_o[:, n * 512:(n + 1) * 512],
                            yT[:, g * SEQ + mt * P:g * SEQ + (mt + 1) * P],
                            wo_sb[:, g * SEQ + n * 512:g * SEQ + (n + 1) * 512],
                            start=(g == 0), stop=(g == NHV - 1))
                nc.vector.tensor_copy(o_sb[:, mt * D:(mt + 1) * D], ps_o[:])
            nc.sync.dma_start(out.rearrange("(m p) n -> p m n", p=P),
                              o_sb[:].rearrange("p (m n) -> p m n", n=D))


def shard_inputs(x, w_q, w_kv, w_o, k_weights):
    """Full inputs -> list of 8 per-core input dicts."""
    scale = 1.0 / math.sqrt(D)
    jj = np.arange(P)[:, None]
    ii = np.arange(IB)[None, :]
    maskq = np.concatenate(
        [(ii >= jj + o * P).astype(np.float32) for o in range(4)], axis=1)
    in_maps = []
    for c in range(N_CORES):
        b, half = c // 2, c % 2
        cols = slice(half * HV, (half + 1) * HV)
        # fold (1 + k_weights)/sqrt(d) into wq's columns
        kw = (1.0 + k_weights[0, half * HL:(half + 1) * HL, 0, :]) * scale
        wq_scaled = w_q[:, cols].astype(np.float64) * kw.reshape(HV)[None, :]
        wqkv = np.concatenate(
            [wq_scaled.astype(np.float32), w_kv[:, cols]], axis=1)
        in_maps.append({
            "xT": np.ascontiguousarray(x[b].T).astype(np.float32),
            "wqkv": np.ascontiguousarray(wqkv),
            "woh": np.ascontiguousarray(w_o[half * HV:(half + 1) * HV, :]),
            "maskq": maskq,
        })
    return in_maps


_CACHED_NC = None


def kernel(x, w_q, w_kv, w_o, k_weights):
    """Full (unsharded) inputs -> full [4, 1024, 1024] output."""
    global _CACHED_NC
    if _CACHED_NC is None:
        _CACHED_NC = build_program()
    nc = _CACHED_NC
    in_maps = shard_inputs(np.asarray(x, dtype=np.float32),
                           np.asarray(w_q, dtype=np.float32),
                           np.asarray(w_kv, dtype=np.float32),
                           np.asarray(w_o, dtype=np.float32),
                           np.asarray(k_weights, dtype=np.float32))
    res = run_bass_kernel_spmd(nc, in_maps, list(range(N_CORES)))
    # each core holds a full-shape partial projection; sum head halves
    outs = [res.results[2 * b]["out"] + res.results[2 * b + 1]["out"]
            for b in range(B)]
    return np.stack(outs, axis=0)


# revision 20
# speedup vs baseline: 2.7761x; 2.7761x over previous
"""Trainium2 Bass kernel for nn_MultiHeadAttention_73504070303932.

Multi-head causal attention with a learned per-head key scale on a shared
K=V projection:

    q  = (x @ w_q) / sqrt(d)          reshaped to (b, h, L, v)
    kv = x @ w_kv                     reshaped to (b, h, L, v)
    k  = kv * (1 + k_weights)
    y  = softmax(causal(q k^T)) @ kv
    out = y @ w_o

Shapes: x [4, 1024, 1024], w_q/w_kv/w_o [1024, 1024],
k_weights [1, 16, 1, 64]; h=16 heads of v=64.

Sharding (8 NeuronCores): data-parallel over batch (4) x tensor-parallel
over head halves (2), with NO device collective: core c handles batch
c//2 and heads (c%2)*8..+8, computes a full-shape PARTIAL output
projection out_c = y_half @ w_o[half rows, :], and the host sums the two
partials per batch while unsharding.  (A previous version exchanged y
halves with an in-kernel AllGather; collectives wedge the device when
re-executed inside a hardware loop, and the host-side add of two
[1024, 1024] partials is negligible.)

Performance model for this target (measured via microbenchmarks): each
engine executes its instruction queue serially; a *freshly streamed*
instruction costs ~50-80 us regardless of size, but re-executing the
same instruction inside a hardware For_i loop costs only ~1-10 us
(f32r matmul N=512 ~4 us, act exp [128,1024] ~6 us, DVE ~1 us; plain
f32 matmuls are ~13x slower than f32r, so everything stays f32r).
Engines run concurrently, so wall time ~= tensor-engine queue time once
streaming is amortized.  The kernel body is therefore built to be
wrapped in a single For_i for repeated execution (timing runs), and the
body itself minimizes tensor-engine instructions (384 matmuls: 128
projection + 96 QK + 96 AV + 64 output projection) and keeps every
other engine off the critical path:

  - projections compute q'^T / kv^T [hv, seq] with weight tiles
    stationary; (1 + k_weights)/sqrt(d) is folded into wq host-side;
    double-buffered 4-bank psum tiles so copy-out overlaps matmuls.
  - kv natural [seq, hv] (the AV stationary operand, with a 65th ones
    column so the softmax denominator accumulates for free in psum row
    64) is produced by a DRAM round trip: one write, one strided
    transpose-gather DMA into the persistent 65-column layout whose
    ones columns are initialized once outside the loop.
  - causal attention per head over 2-j-tile logit groups ([128, 1024]
    psum, double-buffered so the next group's QK matmuls overlap the
    current group's exp on the scalar engine); no max subtraction
    (logits are O(1) by construction); diagonal-straddling groups are
    masked post-exp with a 0/1 mask on the vector engine.
  - softmax normalization per head: reciprocal of psum row 64 (DVE),
    partition-broadcast of the reciprocal row (GpSimd engine, otherwise
    idle), one DVE multiply that also drains the AV psum into y^T.
  - output projection: y^T tiles stationary, w_o moving, 8 psum tiles
    of [128, 1024] rotating through 4 buffers; host adds the two
    partial outputs per batch.

Measured: relative error ~3e-4; per-iteration time (For_i-amortized)
~1.5-2.5 ms/iteration vs ~27 ms for the unrolled baseline.
"""

import math

import numpy as np
from ml_dtypes import bfloat16

import concourse.bass as bass
import concourse.mybir as mybir
import concourse.tile as tile
from concourse import bacc
from concourse.bass_utils import run_bass_kernel_spmd

F32 = mybir.dt.float32
F32R = mybir.dt.float32r
BF16 = mybir.dt.bfloat16

N_CORES = 8
B, SEQ, D = 4, 1024, 1024
H, V = 16, 64
HL = 8          # heads per core
HV = HL * V     # 512 local feature dim
P = 128         # partitions
IB = 512        # i-block (query) width in the attention loop
NI = SEQ // IB  # 2 i-blocks
ND = D // P     # 8 d-tiles
NS = SEQ // P   # 8 seq j-tiles
NHV = HV // P   # 4 local hv tiles
KVC = V + 1     # kv columns per head incl the ones column
GJ = 2          # j-tiles per exp group ([128, GJ*IB] = 2 psum banks)

# set False to replace the GpSimd partition_broadcast in the softmax
# normalization with PE broadcast matmuls (fallback if unsupported)
USE_PBCAST = True


def build_program(n_iters: int = 1, use_loop: bool = True):
    """Build the SPMD program (same for all 8 cores). Returns compiled nc.

    n_iters > 1 repeats the body for on-device timing runs; with
    use_loop=True the repeat is a hardware For_i loop (instructions are
    streamed once and re-executed, ~10x cheaper per iteration than
    streaming fresh instructions).
    """
    nc = bacc.Bacc(trn_type="TRN2", target_bir_lowering=False, debug=False,
                   num_devices=N_CORES)

    xT = nc.dram_tensor("xT", [D, SEQ], BF16, kind="ExternalInput").ap()
    wqkv = nc.dram_tensor("wqkv", [D, 2 * HV], BF16, kind="ExternalInput").ap()
    woh = nc.dram_tensor("woh", [HV, D], BF16, kind="ExternalInput").ap()
    # 0/1 causal masks for the 4 j-tile offsets within an i-block
    maskq = nc.dram_tensor("maskq", [P, 4 * IB], F32, kind="ExternalInput").ap()
    out = nc.dram_tensor("out", [SEQ, D], F32, kind="ExternalOutput").ap()

    Exp = mybir.ActivationFunctionType.Exp

    with tile.TileContext(nc) as tc:
        with (
            tc.tile_pool(name="consts", bufs=1) as consts,
            tc.tile_pool(name="dram", bufs=1, space="DRAM") as dram,
        ):
            maskq_sb = consts.tile([P, 4 * IB], F32)
            ones8_f = consts.tile([P, NS * HL], F32)
            ones64_f = consts.tile([1, V], F32)
            ones64_r = consts.tile([1, V], F32R)
            # kv natural incl. ones columns: persistent so the ones are
            # written once; the per-iteration gather only refreshes the
            # 64 data columns of each (j-tile, head) slot
            kvn = consts.tile([P, NS * HL * KVC], BF16)
            nc.sync.dma_start(maskq_sb[:], maskq[:])
            nc.vector.memset(ones8_f[:], 1.0)
            nc.vector.memset(ones64_f[:], 1.0)
            nc.vector.tensor_copy(ones64_r[:], ones64_f[:])
            nc.vector.tensor_copy(
                kvn[:].rearrange("p (t c) -> p t c", c=KVC)[:, :, V:V + 1],
                ones8_f[:].rearrange("p (t o) -> p t o", o=1))

            kv_dram = dram.tile([HV, SEQ], BF16)

            if use_loop:
                # always a For_i, even for n_iters=1, so 1-iter and N-iter
                # programs have identical static structure and the timing
                # difference is purely N-1 body re-executions
                with tc.For_i(0, n_iters):
                    _one_iter(nc, tc, 0, xT, wqkv, woh, out,
                              maskq_sb, ones64_r, kvn, kv_dram, Exp)
            else:
                for it in range(n_iters):
                    _one_iter(nc, tc, it, xT, wqkv, woh, out,
                              maskq_sb, ones64_r, kvn, kv_dram, Exp)

    nc.compile()
    return nc


def _one_iter(nc, tc, it, xT, wqkv, woh, out,
              maskq_sb, ones64_r, kvn, kv_dram, Exp):
    with (
        tc.tile_pool(name=f"qkv{it}", bufs=1) as qkv,
        tc.tile_pool(name=f"ytp{it}", bufs=1) as ytp,
    ):
        # persistent on-core tensors for this iteration
        qkT = qkv.tile([P, 2 * NHV * SEQ], BF16, tag="qkT", name="qkT")
        qT = qkT[:, 0:NHV * SEQ]
        kvT = qkT[:, NHV * SEQ:2 * NHV * SEQ]
        wo_sb = qkv.tile([P, NHV * SEQ], BF16, tag="wos", name="wos")
        yT = ytp.tile([P, NHV * SEQ], BF16, tag="yTt", name="yTt")

        # ---- load x^T + weights, project q'^T and kv^T ----
        with (
            tc.tile_pool(name=f"xw{it}", bufs=1) as xw,
            tc.tile_pool(name=f"mmps{it}", bufs=2, space="PSUM") as mmps,
        ):
            xT_sb = xw.tile([P, ND * SEQ], BF16, tag="xTs", name="xTs")
            wqkv_sb = xw.tile([P, ND * 2 * HV], BF16, tag="wqs", name="wqs")
            # per-k-tile loads: the m=0 projection chain's k-th matmul only
            # depends on the k-th slices, so PE starts ~one-tile after the
            # first slices land instead of waiting for the full 8 MB
            xT3 = xT.rearrange("(k p) s -> p k s", p=P)
            wq3 = wqkv.rearrange("(k p) n -> p k n", p=P)
            for k in range(ND):
                nc.scalar.dma_start(
                    wqkv_sb[:, k * 2 * HV:(k + 1) * 2 * HV], wq3[:, k, :])
                nc.sync.dma_start(
                    xT_sb[:, k * SEQ:(k + 1) * SEQ], xT3[:, k, :])

            # q^T / kv^T: [hv-tile m, seq] = sum_k w[:, m]^T @ x^T;
            # (1+k_weights)/sqrt(d) is pre-folded into wq's columns
            for m in range(NHV):
                ps_qk = mmps.tile([P, 2048], F32, tag="mm", name="ps_qk")
                for n in range(SEQ // 512):
                    x_k0 = None
                    for k in range(ND):
                        x_k = xT_sb[:, k * SEQ + n * 512:k * SEQ + (n + 1) * 512]
                        wq_k = wqkv_sb[:, k * 2 * HV + m * P:
                                       k * 2 * HV + (m + 1) * P]
                        wkv_k = wqkv_sb[:, k * 2 * HV + HV + m * P:
                                        k * 2 * HV + HV + (m + 1) * P]
                        nc.tensor.matmul(ps_qk[:, n * 512:(n + 1) * 512],
                                         wq_k, x_k,
                                         start=(k == 0), stop=(k == ND - 1))
                        nc.tensor.matmul(
                            ps_qk[:, 1024 + n * 512:1024 + (n + 1) * 512],
                            wkv_k, x_k,
                            start=(k == 0), stop=(k == ND - 1))
                # one copy lands this m-tile's q and kv chunks
                nc.vector.tensor_copy(
                    qkT[:].rearrange("p (sel m s) -> p sel m s",
                                     sel=2, s=SEQ)[:, :, m, :],
                    ps_qk[:].rearrange("p (sel s) -> p sel s", s=SEQ))

        # wo is only needed by the output projection: load it on the ACT
        # engine's DMA queue after the wqkv tiles, overlapping attention
        nc.scalar.dma_start(wo_sb[:].rearrange("p (g n) -> p g n", n=D),
                            woh.rearrange("(g p) n -> p g n", p=P))

        # kv natural via DRAM round trip: write kv^T once, one strided
        # transpose-gather refreshing kvn's 64 data columns per slot
        nc.sync.dma_start(kv_dram.rearrange("(m p) s -> p m s", p=P),
                          kvT[:].rearrange("p (m s) -> p m s", s=SEQ))
        with tc.tile_pool(name=f"kvs{it}", bufs=1) as kvs:
            kvst = kvs.tile([P, NS * HV], BF16, tag="kvst", name="kvst")
            for t in range(NS):
                nc.sync.dma_start(
                    kvst[:, t * HV:(t + 1) * HV],
                    bass.AP(kv_dram.tensor, kv_dram.offset + t * P,
                            [[1, P], [SEQ, HV]]))
            nc.vector.tensor_copy(
                kvn[:].rearrange("p (t h c) -> p t h c",
                                 h=HL, c=KVC)[:, :, :, 0:V],
                kvst[:].rearrange("p (t h c) -> p t h c", h=HL, c=V))

        # ---- causal attention, head by head ----
        with (
            tc.tile_pool(name=f"st{it}", bufs=5) as stp,
            tc.tile_pool(name=f"sd{it}", bufs=2) as sdp,
            tc.tile_pool(name=f"rs{it}", bufs=4) as rsp,
            tc.tile_pool(name=f"ltps{it}", bufs=2, space="PSUM") as ltps,
            tc.tile_pool(name=f"yps{it}", bufs=2, space="PSUM") as yps,
        ):
            for u in range(HL // 2):         # head pair (2u, 2u+1)
                for hh in range(2):
                    h, r0 = 2 * u + hh, hh * V
                    ps_y = yps.tile([V + 1, NI * IB], F32, tag="y",
                                    name="ps_y")
                    sts = {}
                    for i in range(NI):
                        nj = (i + 1) * IB // P   # causal j-tiles
                        for g in range(nj // GJ):
                            ps_l = ltps.tile([P, GJ * IB], F32, tag="lt",
                                             name="ps_l")
                            for jo in range(GJ):
                                j = GJ * g + jo
                                nc.tensor.matmul(
                                    ps_l[:, jo * IB:(jo + 1) * IB],
                                    kvT[r0:r0 + V,
                                        u * SEQ + j * P:u * SEQ + (j + 1) * P],
                                    qT[r0:r0 + V,
                                       u * SEQ + i * IB:u * SEQ + (i + 1) * IB],
                                    start=True, stop=True)
                            st = stp.tile([P, GJ * IB], BF16, tag="st",
                                          name="st")
                            if g >= 2 * i:   # diagonal-straddling group
                                v = g - 2 * i
                                sd = sdp.tile([P, GJ * IB], F32, tag="sd",
                                              name="sd")
                                nc.scalar.activation(sd[:], ps_l[:], Exp)
                                nc.vector.tensor_tensor(
                                    st[:], sd[:],
                                    maskq_sb[:, v * GJ * IB:(v + 1) * GJ * IB],
                                    mybir.AluOpType.mult)
                            else:
                                nc.scalar.activation(st[:], ps_l[:], Exp)
                            sts[(i, g)] = st
                        for j in range(nj):
                            nc.tensor.matmul(
                                ps_y[0:V + 1, i * IB:(i + 1) * IB],
                                kvn[:, (j * HL + h) * KVC:
                                    (j * HL + h + 1) * KVC],
                                sts[(i, j // GJ)][:, (j % GJ) * IB:
                                                  (j % GJ + 1) * IB],
                                start=(j == 0), stop=(j == nj - 1))
                    # normalize: recip of the denominator row, broadcast
                    # across the 64 v-partitions on GpSimd, one multiply
                    # that also drains the AV psum into y^T
                    rr = rsp.tile([1, NI * IB], F32, tag="rr", name="rr")
                    with nc.allow_low_precision(reason="denom to f32"):
                        nc.vector.reciprocal(rr[:], ps_y[V:V + 1, :])
                    rB = rsp.tile([V, NI * IB], F32, tag="rB", name="rB")
                    if USE_PBCAST:
                        nc.gpsimd.partition_broadcast(rB[:], rr[:])
                    else:
                        with tc.tile_pool(name=f"b{it}_{h}", bufs=1,
                                          space="PSUM") as bps:
                            ps_b = bps.tile([V, NI * IB], F32, tag="b",
                                            name="ps_b")
                            for i in range(NI):
                                nc.tensor.matmul(ps_b[:, i * IB:(i + 1) * IB],
                                                 ones64_r[:],
                                                 rr[:, i * IB:(i + 1) * IB],
                                                 start=True, stop=True)
                            nc.vector.tensor_copy(rB[:], ps_b[:])
                    nc.vector.tensor_tensor(
                        yT[r0:r0 + V, u * SEQ:(u + 1) * SEQ],
                        ps_y[0:V, :], rB[:], mybir.AluOpType.mult)

        # ---- partial output projection: out = y_half^T.T @ wo_half ----
        with (
            tc.tile_pool(name=f"os{it}", bufs=1) as osp,
            tc.tile_pool(name=f"ops{it}", bufs=4, space="PSUM") as ops,
        ):
            o_sb = osp.tile([P, NS * D], F32, tag="osb", name="osb")
            for mt in range(NS):
                ps_o = ops.tile([P, D], F32, tag="om", name="ps_o")
                for n in range(D // 512):
                    for g in range(NHV):
                        nc.tensor.matmul(
                            ps_o[:, n * 512:(n + 1) * 512],
                            yT[:, g * SEQ + mt * P:g * SEQ + (mt + 1) * P],
                            wo_sb[:, g * SEQ + n * 512:g * SEQ + (n + 1) * 512],
                            start=(g == 0), stop=(g == NHV - 1))
                nc.vector.tensor_copy(o_sb[:, mt * D:(mt + 1) * D], ps_o[:])
            nc.sync.dma_start(out.rearrange("(m p) n -> p m n", p=P),
                              o_sb[:].rearrange("p (m n) -> p m n", n=D))


def shard_inputs(x, w_q, w_kv, w_o, k_weights):
    """Full inputs -> list of 8 per-core input dicts."""
    scale = 1.0 / math.sqrt(D)
    jj = np.arange(P)[:, None]
    ii = np.arange(IB)[None, :]
    maskq = np.concatenate(
        [(ii >= jj + o * P).astype(np.float32) for o in range(4)], axis=1)
    in_maps = []
    for c in range(N_CORES):
        b, half = c // 2, c % 2
        cols = slice(half * HV, (half + 1) * HV)
        # fold (1 + k_weights)/sqrt(d) into wq's columns
        kw = (1.0 + k_weights[0, half * HL:(half + 1) * HL, 0, :]) * scale
        wq_scaled = w_q[:, cols].astype(np.float64) * kw.reshape(HV)[None, :]
        wqkv = np.concatenate(
            [wq_scaled.astype(np.float32), w_kv[:, cols]], axis=1)
        in_maps.append({
            "xT": np.ascontiguousarray(x[b].T).astype(bfloat16),
            "wqkv": np.ascontiguousarray(wqkv).astype(bfloat16),
            "woh": np.ascontiguousarray(
                w_o[half * HV:(half + 1) * HV, :]).astype(bfloat16),
            "maskq": maskq,
        })
    return in_maps


_CACHED_NC = None


def kernel(x, w_q, w_kv, w_o, k_weights):
    """Full (unsharded) inputs -> full [4, 1024, 1024] output."""
    global _CACHED_NC
    if _CACHED_NC is None:
        _CACHED_NC = build_program()
    nc = _CACHED_NC
    in_maps = shard_inputs(np.asarray(x, dtype=np.float32),
                           np.asarray(w_q, dtype=np.float32),
                           np.asarray(w_kv, dtype=np.float32),
                           np.asarray(w_o, dtype=np.float32),
                           np.asarray(k_weights, dtype=np.float32))
    res = run_bass_kernel_spmd(nc, in_maps, list(range(N_CORES)))
    # each core holds a full-shape partial projection; sum head halves
    outs = [res.results[2 * b]["out"] + res.results[2 * b + 1]["out"]
            for b in range(B)]
    return np.stack(outs, axis=0)


# revision 22
# speedup vs baseline: 3.8011x; 1.3692x over previous
"""Trainium2 Bass kernel for nn_MultiHeadAttention_73504070303932.

Multi-head causal attention with a learned per-head key scale on a shared
K=V projection:

    q  = (x @ w_q) / sqrt(d)          reshaped to (b, h, L, v)
    kv = x @ w_kv                     reshaped to (b, h, L, v)
    k  = kv * (1 + k_weights)
    y  = softmax(causal(q k^T)) @ kv
    out = y @ w_o

Shapes: x [4, 1024, 1024], w_q/w_kv/w_o [1024, 1024],
k_weights [1, 16, 1, 64]; h=16 heads of v=64.

Sharding (8 NeuronCores): data-parallel over batch (4) x tensor-parallel
over head halves (2), with NO device collective: core c handles batch
c//2 and heads (c%2)*8..+8, computes a full-shape PARTIAL output
projection out_c = y_half @ w_o[half rows, :], and the host sums the two
partials per batch while unsharding.  (A previous version exchanged y
halves with an in-kernel AllGather; collectives wedge the device when
re-executed inside a hardware loop, and the host-side add of two
[1024, 1024] partials is negligible.)

Performance model for this target (measured via microbenchmarks): each
engine executes its instruction queue serially; a *freshly streamed*
instruction costs ~50-80 us regardless of size, but re-executing the
same instruction inside a hardware For_i loop costs only ~1-10 us, flat
per instruction (N=128 and N=512 matmuls cost the same; bf16 matmuls
~0.65x of f32r; plain f32 matmuls ~13x slower than f32r).  Engines run
concurrently, so wall time ~= tensor-engine queue time once streaming
is amortized.  The kernel body is therefore built to be wrapped in a
single For_i for repeated execution (timing runs), the whole matmul
path runs in bf16 (inputs are pre-rounded host-side; psum accumulation
stays f32; measured rel err 3.9e-3 vs the 2e-2 gate), and the body
minimizes tensor-engine instructions (384 matmuls: 128 projection + 96
QK + 96 AV + 64 output projection — the floor for M<=128/N<=512/K<=128
per-matmul limits) while keeping every other engine off the critical
path:

  - projections compute q'^T / kv^T [hv, seq] with weight tiles
    stationary; (1 + k_weights)/sqrt(d) is folded into wq host-side;
    double-buffered 4-bank psum tiles so copy-out overlaps matmuls.
  - kv natural [seq, hv] (the AV stationary operand, with a 65th ones
    column so the softmax denominator accumulates for free in psum row
    64) is produced by a DRAM round trip: one write, one strided
    transpose-gather DMA into the persistent 65-column layout whose
    ones columns are initialized once outside the loop.
  - causal attention per head over 2-j-tile logit groups ([128, 1024]
    psum, double-buffered so the next group's QK matmuls overlap the
    current group's exp on the scalar engine); no max subtraction
    (logits are O(1) by construction); diagonal-straddling groups are
    masked post-exp with a 0/1 mask on the vector engine.
  - softmax normalization per head: reciprocal of psum row 64 (DVE),
    partition-broadcast of the reciprocal row (GpSimd engine, otherwise
    idle), one DVE multiply that also drains the AV psum into y^T.
  - output projection: y^T tiles stationary, w_o moving, 8 psum tiles
    of [128, 1024] rotating through 4 buffers; host adds the two
    partial outputs per batch.

Measured: relative error 3.9e-3; per-iteration time (For_i-amortized)
~1.3-1.9 ms vs ~27 ms for the unrolled f32r baseline (load-dependent).
"""

import math

import numpy as np
from ml_dtypes import bfloat16

import concourse.bass as bass
import concourse.mybir as mybir
import concourse.tile as tile
from concourse import bacc
from concourse.bass_utils import run_bass_kernel_spmd

F32 = mybir.dt.float32
F32R = mybir.dt.float32r
BF16 = mybir.dt.bfloat16

N_CORES = 8
B, SEQ, D = 4, 1024, 1024
H, V = 16, 64
HL = 8          # heads per core
HV = HL * V     # 512 local feature dim
P = 128         # partitions
IB = 512        # i-block (query) width in the attention loop
NI = SEQ // IB  # 2 i-blocks
ND = D // P     # 8 d-tiles
NS = SEQ // P   # 8 seq j-tiles
NHV = HV // P   # 4 local hv tiles
KVC = V + 1     # kv columns per head incl the ones column
GJ = 2          # j-tiles per exp group ([128, GJ*IB] = 2 psum banks)

# set False to replace the GpSimd partition_broadcast in the softmax
# normalization with PE broadcast matmuls (fallback if unsupported)
USE_PBCAST = True


def build_program(n_iters: int = 1, use_loop: bool = True):
    """Build the SPMD program (same for all 8 cores). Returns compiled nc.

    n_iters > 1 repeats the body for on-device timing runs; with
    use_loop=True the repeat is a hardware For_i loop (instructions are
    streamed once and re-executed, ~10x cheaper per iteration than
    streaming fresh instructions).
    """
    nc = bacc.Bacc(trn_type="TRN2", target_bir_lowering=False, debug=False,
                   num_devices=N_CORES)

    xT = nc.dram_tensor("xT", [D, SEQ], BF16, kind="ExternalInput").ap()
    wqkv = nc.dram_tensor("wqkv", [D, 2 * HV], BF16, kind="ExternalInput").ap()
    woh = nc.dram_tensor("woh", [HV, D], BF16, kind="ExternalInput").ap()
    # 0/1 causal masks for the 4 j-tile offsets within an i-block
    maskq = nc.dram_tensor("maskq", [P, 4 * IB], F32, kind="ExternalInput").ap()
    out = nc.dram_tensor("out", [SEQ, D], F32, kind="ExternalOutput").ap()

    Exp = mybir.ActivationFunctionType.Exp

    with tile.TileContext(nc) as tc:
        with (
            tc.tile_pool(name="consts", bufs=1) as consts,
            tc.tile_pool(name="dram", bufs=1, space="DRAM") as dram,
        ):
            maskq_sb = consts.tile([P, 4 * IB], F32)
            ones8_f = consts.tile([P, NS * HL], F32)
            ones64_f = consts.tile([1, V], F32)
            ones64_r = consts.tile([1, V], F32R)
            # kv natural incl. ones columns: persistent so the ones are
            # written once; the per-iteration gather only refreshes the
            # 64 data columns of each (j-tile, head) slot
            kvn = consts.tile([P, NS * HL * KVC], BF16)
            nc.sync.dma_start(maskq_sb[:], maskq[:])
            nc.vector.memset(ones8_f[:], 1.0)
            nc.vector.memset(ones64_f[:], 1.0)
            nc.vector.tensor_copy(ones64_r[:], ones64_f[:])
            nc.vector.tensor_copy(
                kvn[:].rearrange("p (t c) -> p t c", c=KVC)[:, :, V:V + 1],
                ones8_f[:].rearrange("p (t o) -> p t o", o=1))

            kv_dram = dram.tile([HV, SEQ], BF16)

            if use_loop:
                # always a For_i, even for n_iters=1, so 1-iter and N-iter
                # programs have identical static structure and the timing
                # difference is purely N-1 body re-executions
                with tc.For_i(0, n_iters):
                    _one_iter(nc, tc, 0, xT, wqkv, woh, out,
                              maskq_sb, ones64_r, kvn, kv_dram, Exp)
            else:
                for it in range(n_iters):
                    _one_iter(nc, tc, it, xT, wqkv, woh, out,
                              maskq_sb, ones64_r, kvn, kv_dram, Exp)

    nc.compile()
    return nc


def _one_iter(nc, tc, it, xT, wqkv, woh, out,
              maskq_sb, ones64_r, kvn, kv_dram, Exp):
    with (
        tc.tile_pool(name=f"qkv{it}", bufs=1) as qkv,
        tc.tile_pool(name=f"ytp{it}", bufs=1) as ytp,
    ):
        # persistent on-core tensors for this iteration
        qkT = qkv.tile([P, 2 * NHV * SEQ], BF16, tag="qkT", name="qkT")
        qT = qkT[:, 0:NHV * SEQ]
        kvT = qkT[:, NHV * SEQ:2 * NHV * SEQ]
        wo_sb = qkv.tile([P, NHV * SEQ], BF16, tag="wos", name="wos")
        yT = ytp.tile([P, NHV * SEQ], BF16, tag="yTt", name="yTt")

        # ---- load x^T + weights, project q'^T and kv^T ----
        with (
            tc.tile_pool(name=f"xw{it}", bufs=1) as xw,
            tc.tile_pool(name=f"mmps{it}", bufs=2, space="PSUM") as mmps,
        ):
            xT_sb = xw.tile([P, ND * SEQ], BF16, tag="xTs", name="xTs")
            wqkv_sb = xw.tile([P, ND * 2 * HV], BF16, tag="wqs", name="wqs")
            # per-k-tile loads: the m=0 projection chain's k-th matmul only
            # depends on the k-th slices, so PE starts ~one-tile after the
            # first slices land instead of waiting for the full 8 MB
            xT3 = xT.rearrange("(k p) s -> p k s", p=P)
            wq3 = wqkv.rearrange("(k p) n -> p k n", p=P)
            for k in range(ND):
                nc.scalar.dma_start(
                    wqkv_sb[:, k * 2 * HV:(k + 1) * 2 * HV], wq3[:, k, :])
                nc.sync.dma_start(
                    xT_sb[:, k * SEQ:(k + 1) * SEQ], xT3[:, k, :])

            # q^T / kv^T: [hv-tile m, seq] = sum_k w[:, m]^T @ x^T;
            # (1+k_weights)/sqrt(d) is pre-folded into wq's columns
            for m in range(NHV):
                ps_qk = mmps.tile([P, 2048], F32, tag="mm", name="ps_qk")
                for n in range(SEQ // 512):
                    x_k0 = None
                    for k in range(ND):
                        x_k = xT_sb[:, k * SEQ + n * 512:k * SEQ + (n + 1) * 512]
                        wq_k = wqkv_sb[:, k * 2 * HV + m * P:
                                       k * 2 * HV + (m + 1) * P]
                        wkv_k = wqkv_sb[:, k * 2 * HV + HV + m * P:
                                        k * 2 * HV + HV + (m + 1) * P]
                        nc.tensor.matmul(ps_qk[:, n * 512:(n + 1) * 512],
                                         wq_k, x_k,
                                         start=(k == 0), stop=(k == ND - 1))
                        nc.tensor.matmul(
                            ps_qk[:, 1024 + n * 512:1024 + (n + 1) * 512],
                            wkv_k, x_k,
                            start=(k == 0), stop=(k == ND - 1))
                # one copy lands this m-tile's q and kv chunks
                nc.vector.tensor_copy(
                    qkT[:].rearrange("p (sel m s) -> p sel m s",
                                     sel=2, s=SEQ)[:, :, m, :],
                    ps_qk[:].rearrange("p (sel s) -> p sel s", s=SEQ))

        # wo is only needed by the output projection: load it on the ACT
        # engine's DMA queue after the wqkv tiles, overlapping attention
        nc.scalar.dma_start(wo_sb[:].rearrange("p (g n) -> p g n", n=D),
                            woh.rearrange("(g p) n -> p g n", p=P))

        # kv natural via DRAM round trip: write kv^T once, one strided
        # transpose-gather refreshing kvn's 64 data columns per slot
        nc.sync.dma_start(kv_dram.rearrange("(m p) s -> p m s", p=P),
                          kvT[:].rearrange("p (m s) -> p m s", s=SEQ))
        with tc.tile_pool(name=f"kvs{it}", bufs=1) as kvs:
            kvst = kvs.tile([P, NS * HV], BF16, tag="kvst", name="kvst")
            for t in range(NS):
                nc.sync.dma_start(
                    kvst[:, t * HV:(t + 1) * HV],
                    bass.AP(kv_dram.tensor, kv_dram.offset + t * P,
                            [[1, P], [SEQ, HV]]))
            nc.vector.tensor_copy(
                kvn[:].rearrange("p (t h c) -> p t h c",
                                 h=HL, c=KVC)[:, :, :, 0:V],
                kvst[:].rearrange("p (t h c) -> p t h c", h=HL, c=V))

        # ---- causal attention, head by head ----
        with (
            tc.tile_pool(name=f"st{it}", bufs=5) as stp,
            tc.tile_pool(name=f"sd{it}", bufs=2) as sdp,
            tc.tile_pool(name=f"rs{it}", bufs=4) as rsp,
            tc.tile_pool(name=f"ltps{it}", bufs=2, space="PSUM") as ltps,
            tc.tile_pool(name=f"yps{it}", bufs=2, space="PSUM") as yps,
        ):
            for u in range(HL // 2):         # head pair (2u, 2u+1)
                for hh in range(2):
                    h, r0 = 2 * u + hh, hh * V
                    ps_y = yps.tile([V + 1, NI * IB], F32, tag="y",
                                    name="ps_y")
                    sts = {}
                    for i in range(NI):
                        nj = (i + 1) * IB // P   # causal j-tiles
                        for g in range(nj // GJ):
                            ps_l = ltps.tile([P, GJ * IB], F32, tag="lt",
                                             name="ps_l")
                            for jo in range(GJ):
                                j = GJ * g + jo
                                nc.tensor.matmul(
                                    ps_l[:, jo * IB:(jo + 1) * IB],
                                    kvT[r0:r0 + V,
                                        u * SEQ + j * P:u * SEQ + (j + 1) * P],
                                    qT[r0:r0 + V,
                                       u * SEQ + i * IB:u * SEQ + (i + 1) * IB],
                                    start=True, stop=True)
                            st = stp.tile([P, GJ * IB], BF16, tag="st",
                                          name="st")
                            if g >= 2 * i:   # diagonal-straddling group
                                v = g - 2 * i
                                sd = sdp.tile([P, GJ * IB], F32, tag="sd",
                                              name="sd")
                                nc.scalar.activation(sd[:], ps_l[:], Exp)
                                nc.vector.tensor_tensor(
                                    st[:], sd[:],
                                    maskq_sb[:, v * GJ * IB:(v + 1) * GJ * IB],
                                    mybir.AluOpType.mult)
                            else:
                                nc.scalar.activation(st[:], ps_l[:], Exp)
                            sts[(i, g)] = st
                        for j in range(nj):
                            nc.tensor.matmul(
                                ps_y[0:V + 1, i * IB:(i + 1) * IB],
                                kvn[:, (j * HL + h) * KVC:
                                    (j * HL + h + 1) * KVC],
                                sts[(i, j // GJ)][:, (j % GJ) * IB:
                                                  (j % GJ + 1) * IB],
                                start=(j == 0), stop=(j == nj - 1))
                    # normalize: recip of the denominator row, broadcast
                    # across the 64 v-partitions on GpSimd, one multiply
                    # that also drains the AV psum into y^T
                    rr = rsp.tile([1, NI * IB], F32, tag="rr", name="rr")
                    with nc.allow_low_precision(reason="denom to f32"):
                        nc.vector.reciprocal(rr[:], ps_y[V:V + 1, :])
                    rB = rsp.tile([V, NI * IB], F32, tag="rB", name="rB")
                    if USE_PBCAST:
                        nc.gpsimd.partition_broadcast(rB[:], rr[:])
                    else:
                        with tc.tile_pool(name=f"b{it}_{h}", bufs=1,
                                          space="PSUM") as bps:
                            ps_b = bps.tile([V, NI * IB], F32, tag="b",
                                            name="ps_b")
                            for i in range(NI):
                                nc.tensor.matmul(ps_b[:, i * IB:(i + 1) * IB],
                                                 ones64_r[:],
                                                 rr[:, i * IB:(i + 1) * IB],
                                                 start=True, stop=True)
                            nc.vector.tensor_copy(rB[:], ps_b[:])
                    nc.vector.tensor_tensor(
                        yT[r0:r0 + V, u * SEQ:(u + 1) * SEQ],
                        ps_y[0:V, :], rB[:], mybir.AluOpType.mult)

        # ---- partial output projection: out = y_half^T.T @ wo_half ----
        with (
            tc.tile_pool(name=f"os{it}", bufs=1) as osp,
            tc.tile_pool(name=f"ops{it}", bufs=4, space="PSUM") as ops,
        ):
            o_sb = osp.tile([P, NS * D], F32, tag="osb", name="osb")
            for mt in range(NS):
                ps_o = ops.tile([P, D], F32, tag="om", name="ps_o")
                for n in range(D // 512):
                    for g in range(NHV):
                        nc.tensor.matmul(
                            ps_o[:, n * 512:(n + 1) * 512],
                            yT[:, g * SEQ + mt * P:g * SEQ + (mt + 1) * P],
                            wo_sb[:, g * SEQ + n * 512:g * SEQ + (n + 1) * 512],
                            start=(g == 0), stop=(g == NHV - 1))
                nc.vector.tensor_copy(o_sb[:, mt * D:(mt + 1) * D], ps_o[:])
            nc.sync.dma_start(out.rearrange("(m p) n -> p m n", p=P),
                              o_sb[:].rearrange("p (m n) -> p m n", n=D))


def shard_inputs(x, w_q, w_kv, w_o, k_weights):
    """Full inputs -> list of 8 per-core input dicts."""
    scale = 1.0 / math.sqrt(D)
    jj = np.arange(P)[:, None]
    ii = np.arange(IB)[None, :]
    maskq = np.concatenate(
        [(ii >= jj + o * P).astype(np.float32) for o in range(4)], axis=1)
    in_maps = []
    for c in range(N_CORES):
        b, half = c // 2, c % 2
        cols = slice(half * HV, (half + 1) * HV)
        # fold (1 + k_weights)/sqrt(d) into wq's columns
        kw = (1.0 + k_weights[0, half * HL:(half + 1) * HL, 0, :]) * scale
        wq_scaled = w_q[:, cols].astype(np.float64) * kw.reshape(HV)[None, :]
        wqkv = np.concatenate(
            [wq_scaled.astype(np.float32), w_kv[:, cols]], axis=1)
        in_maps.append({
            "xT": np.ascontiguousarray(x[b].T).astype(bfloat16),
            "wqkv": np.ascontiguousarray(wqkv).astype(bfloat16),
            "woh": np.ascontiguousarray(
                w_o[half * HV:(half + 1) * HV, :]).astype(bfloat16),
            "maskq": maskq,
        })
    return in_maps


_CACHED_NC = None


def kernel(x, w_q, w_kv, w_o, k_weights):
    """Full (unsharded) inputs -> full [4, 1024, 1024] output."""
    global _CACHED_NC
    if _CACHED_NC is None:
        _CACHED_NC = build_program()
    nc = _CACHED_NC
    in_maps = shard_inputs(np.asarray(x, dtype=np.float32),
                           np.asarray(w_q, dtype=np.float32),
                           np.asarray(w_kv, dtype=np.float32),
                           np.asarray(w_o, dtype=np.float32),
                           np.asarray(k_weights, dtype=np.float32))
    res = run_bass_kernel_spmd(nc, in_maps, list(range(N_CORES)))
    # each core holds a full-shape partial projection; sum head halves
    outs = [res.results[2 * b]["out"] + res.results[2 * b + 1]["out"]
            for b in range(B)]
    return np.stack(outs, axis=0)


# revision 23
# speedup vs baseline: 4.1635x; 1.0953x over previous
"""Trainium2 Bass kernel for nn_MultiHeadAttention_73504070303932.

Multi-head causal attention with a learned per-head key scale on a shared
K=V projection:

    q  = (x @ w_q) / sqrt(d)          reshaped to (b, h, L, v)
    kv = x @ w_kv                     reshaped to (b, h, L, v)
    k  = kv * (1 + k_weights)
    y  = softmax(causal(q k^T)) @ kv
    out = y @ w_o

Shapes: x [4, 1024, 1024], w_q/w_kv/w_o [1024, 1024],
k_weights [1, 16, 1, 64]; h=16 heads of v=64.

Sharding (8 NeuronCores): data-parallel over batch (4) x tensor-parallel
over head halves (2), with NO device collective: core c handles batch
c//2 and heads (c%2)*8..+8, computes a full-shape PARTIAL output
projection out_c = y_half @ w_o[half rows, :], and the host sums the two
partials per batch while unsharding.  (A previous version exchanged y
halves with an in-kernel AllGather; collectives wedge the device when
re-executed inside a hardware loop, and the host-side add of two
[1024, 1024] partials is negligible.)

Performance model for this target (measured via microbenchmarks): each
engine executes its instruction queue serially; a *freshly streamed*
instruction costs ~50-80 us regardless of size, but re-executing the
same instruction inside a hardware For_i loop costs only ~1-10 us, flat
per instruction (N=128 and N=512 matmuls cost the same; bf16 matmuls
~0.65x of f32r; plain f32 matmuls ~13x slower than f32r).  Engines run
concurrently, so wall time ~= tensor-engine queue time once streaming
is amortized.  The kernel body is therefore built to be wrapped in a
single For_i for repeated execution (timing runs), the whole matmul
path runs in bf16 (inputs are pre-rounded host-side; psum accumulation
stays f32; measured rel err 3.9e-3 vs the 2e-2 gate), and the body
minimizes tensor-engine instructions (384 matmuls: 128 projection + 96
QK + 96 AV + 64 output projection — the floor for M<=128/N<=512/K<=128
per-matmul limits) while keeping every other engine off the critical
path:

  - projections compute q'^T / kv^T [hv, seq] with weight tiles
    stationary; (1 + k_weights)/sqrt(d) is folded into wq host-side;
    double-buffered 4-bank psum tiles so copy-out overlaps matmuls.
  - kv natural [seq, hv] (the AV stationary operand, with a 65th ones
    column so the softmax denominator accumulates for free in psum row
    64) is produced by a DRAM round trip: one write, one strided
    transpose-gather DMA into the persistent 65-column layout whose
    ones columns are initialized once outside the loop.
  - causal attention per head over 2-j-tile logit groups ([128, 1024]
    psum, double-buffered so the next group's QK matmuls overlap the
    current group's exp on the scalar engine); no max subtraction
    (logits are O(1) by construction); diagonal-straddling groups are
    masked post-exp with a 0/1 mask on the vector engine.
  - softmax normalization per head: reciprocal of psum row 64 (DVE),
    partition-broadcast of the reciprocal row (GpSimd engine, otherwise
    idle), one DVE multiply that also drains the AV psum into y^T.
  - output projection: y^T tiles stationary, w_o moving, 8 psum tiles
    of [128, 1024] rotating through 4 buffers; host adds the two
    partial outputs per batch.

Measured: relative error 3.9e-3; per-iteration time (For_i-amortized)
~1.3-1.9 ms vs ~27 ms for the unrolled f32r baseline (load-dependent).
"""

import math

import numpy as np
from ml_dtypes import bfloat16

import concourse.bass as bass
import concourse.mybir as mybir
import concourse.tile as tile
from concourse import bacc
from concourse.bass_utils import run_bass_kernel_spmd

F32 = mybir.dt.float32
F32R = mybir.dt.float32r
BF16 = mybir.dt.bfloat16

N_CORES = 8
B, SEQ, D = 4, 1024, 1024
H, V = 16, 64
HL = 8          # heads per core
HV = HL * V     # 512 local feature dim
P = 128         # partitions
IB = 512        # i-block (query) width in the attention loop
NI = SEQ // IB  # 2 i-blocks
ND = D // P     # 8 d-tiles
NS = SEQ // P   # 8 seq j-tiles
NHV = HV // P   # 4 local hv tiles
KVC = V + 1     # kv columns per head incl the ones column
GJ = 2          # j-tiles per exp group ([128, GJ*IB] = 2 psum banks)

# set False to replace the GpSimd partition_broadcast in the softmax
# normalization with PE broadcast matmuls (fallback if unsupported)
USE_PBCAST = True


def build_program(n_iters: int = 1, use_loop: bool = True):
    """Build the SPMD program (same for all 8 cores). Returns compiled nc.

    n_iters > 1 repeats the body for on-device timing runs; with
    use_loop=True the repeat is a hardware For_i loop (instructions are
    streamed once and re-executed, ~10x cheaper per iteration than
    streaming fresh instructions).
    """
    nc = bacc.Bacc(trn_type="TRN2", target_bir_lowering=False, debug=False,
                   num_devices=N_CORES)

    xT = nc.dram_tensor("xT", [D, SEQ], BF16, kind="ExternalInput").ap()
    wqkv = nc.dram_tensor("wqkv", [D, 2 * HV], BF16, kind="ExternalInput").ap()
    woh = nc.dram_tensor("woh", [HV, D], BF16, kind="ExternalInput").ap()
    # 0/1 causal masks for the 4 j-tile offsets within an i-block
    maskq = nc.dram_tensor("maskq", [P, 4 * IB], F32, kind="ExternalInput").ap()
    out = nc.dram_tensor("out", [SEQ, D], F32, kind="ExternalOutput").ap()

    Exp = mybir.ActivationFunctionType.Exp

    with tile.TileContext(nc) as tc:
        with (
            tc.tile_pool(name="consts", bufs=1) as consts,
            tc.tile_pool(name="dram", bufs=1, space="DRAM") as dram,
        ):
            maskq_sb = consts.tile([P, 4 * IB], F32)
            ones8_f = consts.tile([P, NS * HL], F32)
            ones64_f = consts.tile([1, V], F32)
            ones64_r = consts.tile([1, V], F32R)
            # kv natural incl. ones columns: persistent so the ones are
            # written once; the per-iteration gather only refreshes the
            # 64 data columns of each (j-tile, head) slot
            kvn = consts.tile([P, NS * HL * KVC], BF16)
            nc.sync.dma_start(maskq_sb[:], maskq[:])
            nc.vector.memset(ones8_f[:], 1.0)
            nc.vector.memset(ones64_f[:], 1.0)
            nc.vector.tensor_copy(ones64_r[:], ones64_f[:])
            nc.vector.tensor_copy(
                kvn[:].rearrange("p (t c) -> p t c", c=KVC)[:, :, V:V + 1],
                ones8_f[:].rearrange("p (t o) -> p t o", o=1))

            kv_dram = dram.tile([HV, SEQ], BF16)

            if use_loop:
                # always a For_i, even for n_iters=1, so 1-iter and N-iter
                # programs have identical static structure and the timing
                # difference is purely N-1 body re-executions
                with tc.For_i(0, n_iters):
                    _one_iter(nc, tc, 0, xT, wqkv, woh, out,
                              maskq_sb, ones64_r, kvn, kv_dram, Exp)
            else:
                for it in range(n_iters):
                    _one_iter(nc, tc, it, xT, wqkv, woh, out,
                              maskq_sb, ones64_r, kvn, kv_dram, Exp)

    nc.compile()
    return nc


def _one_iter(nc, tc, it, xT, wqkv, woh, out,
              maskq_sb, ones64_r, kvn, kv_dram, Exp):
    with (
        tc.tile_pool(name=f"qkv{it}", bufs=1) as qkv,
        tc.tile_pool(name=f"ytp{it}", bufs=1) as ytp,
    ):
        # persistent on-core tensors for this iteration
        qkT = qkv.tile([P, 2 * NHV * SEQ], BF16, tag="qkT", name="qkT")
        qT = qkT[:, 0:NHV * SEQ]
        kvT = qkT[:, NHV * SEQ:2 * NHV * SEQ]
        wo_sb = qkv.tile([P, NHV * SEQ], BF16, tag="wos", name="wos")
        yT = ytp.tile([P, NHV * SEQ], BF16, tag="yTt", name="yTt")

        # ---- load x^T + weights, project q'^T and kv^T ----
        with (
            tc.tile_pool(name=f"xw{it}", bufs=1) as xw,
            tc.tile_pool(name=f"mmps{it}", bufs=2, space="PSUM") as mmps,
        ):
            xT_sb = xw.tile([P, ND * SEQ], BF16, tag="xTs", name="xTs")
            wqkv_sb = xw.tile([P, ND * 2 * HV], BF16, tag="wqs", name="wqs")
            # per-k-tile loads: the m=0 projection chain's k-th matmul only
            # depends on the k-th slices, so PE starts ~one-tile after the
            # first slices land instead of waiting for the full 8 MB
            xT3 = xT.rearrange("(k p) s -> p k s", p=P)
            wq3 = wqkv.rearrange("(k p) n -> p k n", p=P)
            for k in range(ND):
                nc.scalar.dma_start(
                    wqkv_sb[:, k * 2 * HV:(k + 1) * 2 * HV], wq3[:, k, :])
                nc.sync.dma_start(
                    xT_sb[:, k * SEQ:(k + 1) * SEQ], xT3[:, k, :])

            # q^T / kv^T: [hv-tile m, seq] = sum_k w[:, m]^T @ x^T;
            # (1+k_weights)/sqrt(d) is pre-folded into wq's columns
            for m in range(NHV):
                ps_qk = mmps.tile([P, 2048], F32, tag="mm", name="ps_qk")
                for n in range(SEQ // 512):
                    x_k0 = None
                    for k in range(ND):
                        x_k = xT_sb[:, k * SEQ + n * 512:k * SEQ + (n + 1) * 512]
                        wq_k = wqkv_sb[:, k * 2 * HV + m * P:
                                       k * 2 * HV + (m + 1) * P]
                        wkv_k = wqkv_sb[:, k * 2 * HV + HV + m * P:
                                        k * 2 * HV + HV + (m + 1) * P]
                        nc.tensor.matmul(ps_qk[:, n * 512:(n + 1) * 512],
                                         wq_k, x_k,
                                         start=(k == 0), stop=(k == ND - 1))
                        nc.tensor.matmul(
                            ps_qk[:, 1024 + n * 512:1024 + (n + 1) * 512],
                            wkv_k, x_k,
                            start=(k == 0), stop=(k == ND - 1))
                # one copy lands this m-tile's q and kv chunks
                nc.vector.tensor_copy(
                    qkT[:].rearrange("p (sel m s) -> p sel m s",
                                     sel=2, s=SEQ)[:, :, m, :],
                    ps_qk[:].rearrange("p (sel s) -> p sel s", s=SEQ))

        # wo is only needed by the output projection: load it on the ACT
        # engine's DMA queue after the wqkv tiles, overlapping attention
        nc.scalar.dma_start(wo_sb[:].rearrange("p (g n) -> p g n", n=D),
                            woh.rearrange("(g p) n -> p g n", p=P))

        # kv natural via DRAM round trip: write kv^T once, one strided
        # transpose-gather refreshing kvn's 64 data columns per slot
        nc.sync.dma_start(kv_dram.rearrange("(m p) s -> p m s", p=P),
                          kvT[:].rearrange("p (m s) -> p m s", s=SEQ))
        with tc.tile_pool(name=f"kvs{it}", bufs=1) as kvs:
            kvst = kvs.tile([P, NS * HV], BF16, tag="kvst", name="kvst")
            for t in range(NS):
                nc.sync.dma_start(
                    kvst[:, t * HV:(t + 1) * HV],
                    bass.AP(kv_dram.tensor, kv_dram.offset + t * P,
                            [[1, P], [SEQ, HV]]))
            nc.vector.tensor_copy(
                kvn[:].rearrange("p (t h c) -> p t h c",
                                 h=HL, c=KVC)[:, :, :, 0:V],
                kvst[:].rearrange("p (t h c) -> p t h c", h=HL, c=V))

        # ---- causal attention, head by head ----
        with (
            tc.tile_pool(name=f"st{it}", bufs=9) as stp,
            tc.tile_pool(name=f"sd{it}", bufs=3) as sdp,
            tc.tile_pool(name=f"rs{it}", bufs=6) as rsp,
            tc.tile_pool(name=f"ltps{it}", bufs=2, space="PSUM") as ltps,
            tc.tile_pool(name=f"yps{it}", bufs=2, space="PSUM") as yps,
        ):
            for u in range(HL // 2):         # head pair (2u, 2u+1)
                for hh in range(2):
                    h, r0 = 2 * u + hh, hh * V
                    ps_y = yps.tile([V + 1, NI * IB], F32, tag="y",
                                    name="ps_y")
                    sts = {}
                    for i in range(NI):
                        nj = (i + 1) * IB // P   # causal j-tiles
                        for g in range(nj // GJ):
                            ps_l = ltps.tile([P, GJ * IB], F32, tag="lt",
                                             name="ps_l")
                            for jo in range(GJ):
                                j = GJ * g + jo
                                nc.tensor.matmul(
                                    ps_l[:, jo * IB:(jo + 1) * IB],
                                    kvT[r0:r0 + V,
                                        u * SEQ + j * P:u * SEQ + (j + 1) * P],
                                    qT[r0:r0 + V,
                                       u * SEQ + i * IB:u * SEQ + (i + 1) * IB],
                                    start=True, stop=True)
                            st = stp.tile([P, GJ * IB], BF16, tag="st",
                                          name="st")
                            if g >= 2 * i:   # diagonal-straddling group
                                v = g - 2 * i
                                sd = sdp.tile([P, GJ * IB], F32, tag="sd",
                                              name="sd")
                                nc.scalar.activation(sd[:], ps_l[:], Exp)
                                nc.vector.tensor_tensor(
                                    st[:], sd[:],
                                    maskq_sb[:, v * GJ * IB:(v + 1) * GJ * IB],
                                    mybir.AluOpType.mult)
                            else:
                                nc.scalar.activation(st[:], ps_l[:], Exp)
                            sts[(i, g)] = st
                        for j in range(nj):
                            nc.tensor.matmul(
                                ps_y[0:V + 1, i * IB:(i + 1) * IB],
                                kvn[:, (j * HL + h) * KVC:
                                    (j * HL + h + 1) * KVC],
                                sts[(i, j // GJ)][:, (j % GJ) * IB:
                                                  (j % GJ + 1) * IB],
                                start=(j == 0), stop=(j == nj - 1))
                    # normalize: recip of the denominator row, broadcast
                    # across the 64 v-partitions on GpSimd, one multiply
                    # that also drains the AV psum into y^T
                    rr = rsp.tile([1, NI * IB], F32, tag="rr", name="rr")
                    with nc.allow_low_precision(reason="denom to f32"):
                        nc.vector.reciprocal(rr[:], ps_y[V:V + 1, :])
                    rB = rsp.tile([V, NI * IB], F32, tag="rB", name="rB")
                    if USE_PBCAST:
                        nc.gpsimd.partition_broadcast(rB[:], rr[:])
                    else:
                        with tc.tile_pool(name=f"b{it}_{h}", bufs=1,
                                          space="PSUM") as bps:
                            ps_b = bps.tile([V, NI * IB], F32, tag="b",
                                            name="ps_b")
                            for i in range(NI):
                                nc.tensor.matmul(ps_b[:, i * IB:(i + 1) * IB],
                                                 ones64_r[:],
                                                 rr[:, i * IB:(i + 1) * IB],
                                                 start=True, stop=True)
                            nc.vector.tensor_copy(rB[:], ps_b[:])
                    nc.vector.tensor_tensor(
                        yT[r0:r0 + V, u * SEQ:(u + 1) * SEQ],
                        ps_y[0:V, :], rB[:], mybir.AluOpType.mult)

        # ---- partial output projection: out = y_half^T.T @ wo_half ----
        with (
            tc.tile_pool(name=f"os{it}", bufs=1) as osp,
            tc.tile_pool(name=f"ops{it}", bufs=4, space="PSUM") as ops,
        ):
            o_sb = osp.tile([P, NS * D], F32, tag="osb", name="osb")
            for mt in range(NS):
                ps_o = ops.tile([P, D], F32, tag="om", name="ps_o")
                for n in range(D // 512):
                    for g in range(NHV):
                        nc.tensor.matmul(
                            ps_o[:, n * 512:(n + 1) * 512],
                            yT[:, g * SEQ + mt * P:g * SEQ + (mt + 1) * P],
                            wo_sb[:, g * SEQ + n * 512:g * SEQ + (n + 1) * 512],
                            start=(g == 0), stop=(g == NHV - 1))
                nc.vector.tensor_copy(o_sb[:, mt * D:(mt + 1) * D], ps_o[:])
            nc.sync.dma_start(out.rearrange("(m p) n -> p m n", p=P),
                              o_sb[:].rearrange("p (m n) -> p m n", n=D))


def shard_inputs(x, w_q, w_kv, w_o, k_weights):
    """Full inputs -> list of 8 per-core input dicts."""
    scale = 1.0 / math.sqrt(D)
    jj = np.arange(P)[:, None]
    ii = np.arange(IB)[None, :]
    maskq = np.concatenate(
        [(ii >= jj + o * P).astype(np.float32) for o in range(4)], axis=1)
    in_maps = []
    for c in range(N_CORES):
        b, half = c // 2, c % 2
        cols = slice(half * HV, (half + 1) * HV)
        # fold (1 + k_weights)/sqrt(d) into wq's columns
        kw = (1.0 + k_weights[0, half * HL:(half + 1) * HL, 0, :]) * scale
        wq_scaled = w_q[:, cols].astype(np.float64) * kw.reshape(HV)[None, :]
        wqkv = np.concatenate(
            [wq_scaled.astype(np.float32), w_kv[:, cols]], axis=1)
        in_maps.append({
            "xT": np.ascontiguousarray(x[b].T).astype(bfloat16),
            "wqkv": np.ascontiguousarray(wqkv).astype(bfloat16),
            "woh": np.ascontiguousarray(
                w_o[half * HV:(half + 1) * HV, :]).astype(bfloat16),
            "maskq": maskq,
        })
    return in_maps


_CACHED_NC = None


def kernel(x, w_q, w_kv, w_o, k_weights):
    """Full (unsharded) inputs -> full [4, 1024, 1024] output."""
    global _CACHED_NC
    if _CACHED_NC is None:
        _CACHED_NC = build_program()
    nc = _CACHED_NC
    in_maps = shard_inputs(np.asarray(x, dtype=np.float32),
                           np.asarray(w_q, dtype=np.float32),
                           np.asarray(w_kv, dtype=np.float32),
                           np.asarray(w_o, dtype=np.float32),
                           np.asarray(k_weights, dtype=np.float32))
    res = run_bass_kernel_spmd(nc, in_maps, list(range(N_CORES)))
    # each core holds a full-shape partial projection; sum head halves
    outs = [res.results[2 * b]["out"] + res.results[2 * b + 1]["out"]
            for b in range(B)]
    return np.stack(outs, axis=0)
